# revision 46
# baseline (speedup 1.0000x reference)
"""DCSLoss Trainium2 kernel v2 (8-core SPMD Bass/Tile).

Same math as the baseline (see kernel.py docstring), rebuilt around the
TimelineSim cost model:

- one ACT table load (no DMAs issued from the Activation engine)
- x ships as bf16, pre-rearranged [TP, NT, D] on the host; per-sample
  normalization is folded into the center-selector weights (W' = W * rinv)
  so the [TP, D] normalize pass disappears
- the AllGather output AP is rank-interleaved ([p, r, m] in DRAM) so the
  post-AG rhs load is one contiguous [128 x 2064B] DMA
- sq ships as an fp8 hi/lo pair of (|fp8 cc|^2 + EPS); the eps rides into
  the G matmul so the DVE max() pass disappears
- PE is kept at full pstate through the collective window by a chained
  dummy-matmul stream (cost model: PE ramps to 2.4GHz after 3us busy)
- the output writeback is a pre-prepared SWDGE scatter-add into the
  zero-donated output buffer (saves HWDGE+DGE latency on the tail)
"""

import os
import sys

import numpy as np

sys.path.insert(0, "/opt/trn_rl_repo")

import ml_dtypes  # noqa: E402

import concourse.bacc as bacc  # noqa: E402
import concourse.mybir as mybir  # noqa: E402
import concourse.tile as tile  # noqa: E402
from concourse.bass_utils import run_bass_kernel_spmd  # noqa: E402

N, D, K = 8192, 256, 8
NCORES = 8
NL = N // NCORES
TP = 128
NT = NL // TP
CPT = TP // K
C = N // K
MARGIN2 = 0.7
EPS = 6e-4
AG_COLS = 2 * TP + 2
F32 = mybir.dt.float32
BF16 = mybir.dt.bfloat16
FP8 = mybir.dt.float8e4
I16 = mybir.dt.int16

NDUM = int(os.environ.get("DCS_NDUM", "108"))  # PE warm-keeping dummies
NGAP = int(os.environ.get("DCS_NGAP", "8"))  # fine-grained gap fillers
OUT_COLS = 64  # scatter-add elem: 64 f32 = 256B

_CACHE: dict = {}


def _build_nc():
    AluOp = mybir.AluOpType
    Act = mybir.ActivationFunctionType
    stage = os.environ.get("DCS_STAGE", "all")

    nc = bacc.Bacc(target_bir_lowering=True)
    x = nc.dram_tensor("x", [TP, NT, D], mybir.dt.uint8, kind="ExternalInput")
    wcc = nc.dram_tensor("wcc", [TP, NT, TP], mybir.dt.uint8, kind="ExternalInput")
    wpc = nc.dram_tensor("wpc", [TP, NT, TP], BF16, kind="ExternalInput")
    eyeb = nc.dram_tensor("eyeb", [TP, TP], BF16, kind="ExternalInput")
    out = nc.dram_tensor("partials", [TP, 4], F32, kind="ExternalOutput")

    with tile.TileContext(nc) as tc:
        with (
            tc.tile_pool(name="const", bufs=1) as cp,
            tc.tile_pool(name="work", bufs=2) as wp,
            tc.tile_pool(name="psum", bufs=1, space="PSUM") as pp,
            tc.tile_pool(name="dram", bufs=1, space="DRAM") as dp,
        ):
            # --- constants (DVE memsets; the sqrt pins the ACT table) ---
            ones = cp.tile([TP, 1], F32)
            nc.vector.memset(ones[:], 1.0)
            actp = cp.tile([TP, 1], F32)
            nc.scalar.sqrt(actp[:], ones[:])
            b_m2 = cp.tile([TP, 1], F32)
            nc.vector.memset(b_m2[:], MARGIN2)
            ones_b = cp.tile([1, TP], FP8)
            nc.vector.memset(ones_b[:], 1.0)
            dum_s = cp.tile([TP, TP], FP8)
            nc.vector.memset(dum_s[:], 0.25)
            dum_m = cp.tile([TP, 512], FP8)
            nc.vector.memset(dum_m[:], 0.25)
            fin = cp.tile([TP, OUT_COLS], F32)
            nc.vector.memset(fin[:], 0.0)

            # --- input DMAs. SP carries the latency-critical loads; no DMA
            # ever issues from the scalar engine (it would pull in a second
            # ACT function table load). ---
            Xs = cp.tile([TP, NT, D], mybir.dt.uint8)
            WCC = cp.tile([TP, NT, TP], mybir.dt.uint8)
            nc.sync.dma_start(Xs[:, 0:3, :], x[:, 0:3, :])
            nc.sync.dma_start(WCC[:], wcc[:])
            nc.sync.dma_start(Xs[:, 3:6, :], x[:, 3:6, :])
            nc.sync.dma_start(Xs[:, 6:8, :], x[:, 6:8, :])
            EYE = cp.tile([TP, TP], BF16)
            nc.sync.dma_start(EYE[:], eyeb[:])

            # --- per-chunk normalize-free pipeline: squared norms -> rinv ->
            # scaled selector -> center matmul. All 8 matmuls accumulate into
            # one [TP, D] PSUM tile. ---
            n2 = wp.tile([TP, NT, 1], F32, tag="n2")
            nrm = wp.tile([TP, NT, 1], F32, tag="nrm")
            rinv = wp.tile([TP, NT], F32, tag="rinv")
            ccp = pp.tile([TP, D], F32, tag="ccp")
            chunks = [(0, 3), (3, 6), (6, 8)]
            for q, (t0, t1) in enumerate(chunks):
                ts = slice(t0, t1)
                scr = wp.tile([TP, t1 - t0, D], BF16, tag=f"nsq{q}")
                if q == 2:
                    nc.gpsimd.tensor_tensor(
                        scr[:], Xs[:, ts, :].bitcast(FP8),
                        Xs[:, ts, :].bitcast(FP8), AluOp.mult,
                    )
                else:
                    nc.scalar.activation(
                        scr[:], Xs[:, ts, :].bitcast(FP8), Act.Square
                    )
                nc.vector.tensor_reduce(
                    n2[:, ts, :], scr[:],
                    axis=mybir.AxisListType.X, op=AluOp.add,
                )
                nc.scalar.activation(nrm[:, ts, :], n2[:, ts, :], Act.Sqrt)
                nc.vector.reciprocal(rinv[:, ts], nrm[:, ts, 0])
                for t in range(t0, t1):
                    wt = wp.tile([TP, TP], FP8, tag=f"wt{t % 3}")
                    nc.vector.tensor_scalar_mul(
                        wt[:], WCC[:, t, :].bitcast(FP8), rinv[:, t : t + 1]
                    )
                    nc.tensor.matmul(
                        ccp[:], wt[:], Xs[:, t, :].bitcast(FP8),
                        start=(t == 0), stop=(t == NT - 1),
                    )

            # --- pack: fp8 transposed centers + sq hi/lo (of |fp8 cc|^2+EPS,
            # so fp8 rounding cancels in the hinge and the eps folds in). ---
            ccb = wp.tile([TP, D], BF16, tag="ccb")
            nc.vector.tensor_copy(ccb[:], ccp[:])
            cc8 = wp.tile([TP, D], FP8, tag="cc8")
            nc.scalar.copy(cc8[:], ccb[:])
            sqc = wp.tile([TP, 1], F32, tag="sqc")
            scr2 = wp.tile([TP, D], BF16, tag="scr2")
            nc.scalar.activation(scr2[:], cc8[:], Act.Square, accum_out=sqc[:])

            t12 = pp.tile([TP, 2 * TP], BF16, tag="t12")
            nc.tensor.transpose(t12[:, 0:TP], ccb[:, 0:TP], EYE[:])
            nc.tensor.transpose(t12[:, TP : 2 * TP], ccb[:, TP : 2 * TP], EYE[:])
            ct_ext = wp.tile([TP, 512], FP8, tag="ct_ext")
            nc.scalar.copy(ct_ext[:, 0:TP], t12[:, 0:TP])
            nc.scalar.copy(ct_ext[:, TP : 2 * TP], t12[:, TP : 2 * TP])
            nc.vector.tensor_copy(ct_ext[:, 2 * TP : 2 * TP + 1], sqc[:])
            nc.vector.memset(ct_ext[:, 2 * TP + 1 : AG_COLS], 0.0)

            ag_in = dp.tile([TP, AG_COLS], FP8)
            nc.sync.dma_start(ag_in[:, 0:TP], ct_ext[:, 0:TP])
            nc.sync.dma_start(ag_in[:, TP:AG_COLS], ct_ext[:, TP:AG_COLS])
            # contiguous row-major gather target; 3-D AP passed unopted
            ag_out = dp.tile([NCORES, TP, AG_COLS], FP8, addr_space="Shared")
            if stage != "noag":
                nc.gpsimd.collective_compute(
                    "AllGather",
                    AluOp.bypass,
                    replica_groups=[list(range(NCORES))],
                    ins=[ag_in.opt()],
                    outs=[ag_out[:]],
                )

            # --- G matmul stationaries (fp8, in-window) ---
            l0 = wp.tile([TP, TP], FP8, tag="l0")
            nc.scalar.mul(l0[:], t12[:, 0:TP], -2.0)
            l1 = wp.tile([TP, TP], FP8, tag="l1")
            nc.scalar.mul(l1[:], t12[:, TP : 2 * TP], -2.0)

            # --- window work: dist_pc (exact bf16 path) ---
            WPC = cp.tile([TP, NT, TP], BF16)
            nc.sync.dma_start(WPC[:], wpc[:])

            sqn = wp.tile([TP, 1], F32, tag="sqn")
            nc.scalar.activation(sqn[:], sqc[:], Act.Sqrt)
            rc = wp.tile([TP, 1], F32, tag="rc")
            nc.vector.reciprocal(rc[:], sqn[:])
            cn = wp.tile([TP, D], BF16, tag="cn")
            nc.vector.tensor_scalar_mul(cn[:], ccb[:], rc[:])

            dots = wp.tile([TP, NT], F32, tag="dots")
            for t in range(NT) if stage != "nodpc" else []:
                ep = pp.tile([TP, D], F32, tag="ep", bufs=2)
                nc.tensor.matmul(
                    ep[:], WPC[:, t, :], cn[:], start=True, stop=True
                )
                scr3 = wp.tile([TP, D], BF16, tag="scr3", bufs=4)
                nc.vector.tensor_tensor(
                    scr3[:], Xs[:, t, :].bitcast(FP8), ep[:], AluOp.mult
                )
                nc.vector.tensor_reduce(
                    dots[:, t : t + 1], scr3[:],
                    axis=mybir.AxisListType.X, op=AluOp.add,
                )
            if stage == "nodpc":
                nc.vector.memset(dots[:], 0.5)
            d1 = wp.tile([TP, NT], F32, tag="d1")
            nc.vector.tensor_tensor(d1[:], dots[:], rinv[:], AluOp.mult)
            d2 = wp.tile([TP, NT], F32, tag="d2")
            nc.vector.tensor_scalar(
                d2[:], d1[:], -2.0, 2.0, AluOp.mult, AluOp.add
            )
            d3 = wp.tile([TP, NT], F32, tag="d3")
            nc.vector.tensor_scalar_max(d3[:], d2[:], 0.0)
            dpc = wp.tile([TP, NT], F32, tag="dpc")
            nc.scalar.activation(
                dpc[:], d3[:], Act.Sqrt, accum_out=fin[:, 0:1]
            )

            # --- PE warm-keeping: chained dummy matmuls spanning the
            # collective window so the post-AG matmuls run at full pstate.
            # The first reads l0 so the stream starts after the pack; the
            # result feeds fin col 3 (ignored by the host) to stay live. ---
            dps = pp.tile([TP, 512], F32, tag="dps")
            for i in range(NDUM):
                nc.tensor.matmul(
                    dps[:], (l0 if i == 0 else dum_s)[:],
                    dum_m[:],
                    start=(i == 0), stop=(i == NDUM - 1),
                )
            nc.vector.tensor_reduce(
                fin[:, 3:4], dps[:, 0:2],
                axis=mybir.AxisListType.X, op=AluOp.add,
            )

            # --- post-AG: one contiguous rhs load + hi/lo sq rows, then two
            # pipelined q-halves of [l0, l1, ones] matmuls -> sqrt -> hinge.
            rhs2 = wp.tile([1, NCORES, TP], FP8, tag="rhs2")
            nc.sync.dma_start(
                rhs2[:],
                ag_out[:, :, 2 * TP : 2 * TP + 1].transpose([2, 0, 1]),
            )
            rhs = wp.tile([TP, NCORES, AG_COLS], FP8, tag="rhs")
            nc.sync.dma_start(
                rhs[:, 0:4, :], ag_out[0:4, :, :].transpose([1, 0, 2])
            )
            nc.sync.dma_start(
                rhs[:, 4:8, :], ag_out[4:8, :, :].transpose([1, 0, 2])
            )

            NQ = 2
            RQ = NCORES // NQ
            QW = C // NQ
            for q in range(NQ) if stage != "nog" else []:
                rq = slice(q * RQ, (q + 1) * RQ)
                gp = pp.tile([TP, QW], F32, tag="gp", bufs=2)
                nc.tensor.matmul(
                    gp[:], ones_b[:],
                    rhs2[:, rq, :],
                    start=True, stop=False,
                )
                nc.tensor.matmul(
                    gp[:], l0[:], rhs[:, rq, 0:TP], start=False, stop=False
                )
                nc.tensor.matmul(
                    gp[:], l1[:], rhs[:, rq, TP : 2 * TP],
                    start=False, stop=True,
                )
                tha = wp.tile([TP, QW], F32, tag="tha")
                nc.vector.tensor_scalar(
                    tha[:], gp[:], sqc[:], MARGIN2 * MARGIN2,
                    AluOp.add, AluOp.min,
                )
                th = wp.tile([TP, QW], BF16, tag="th")
                nc.vector.tensor_scalar_max(th[:], tha[:], 0.0)
                sh = wp.tile([TP, QW], BF16, tag="sh")
                nc.scalar.activation(
                    sh[:], th[:], Act.Sqrt, accum_out=fin[:, 1 + q : 2 + q]
                )
            if stage == "nog":
                nc.vector.memset(fin[:, 1:3], 0.0)

            nc.sync.dma_start(out[:], fin[:, 0:4])

    nc.finalize()
    return nc


def _aux_inputs() -> dict:
    s = np.arange(TP)
    wcc = np.zeros((TP, NT, TP), ml_dtypes.float8_e4m3fn)  # shipped as uint8 bits
    wpc = np.zeros((TP, NT, TP), ml_dtypes.bfloat16)
    for t in range(NT):
        wcc[s, t, t * CPT + s // K] = 1.0 / K
        wpc[t * CPT + s // K, t, s] = 1.0
    eyeb = np.eye(TP, dtype=ml_dtypes.bfloat16)
    return {"wcc": wcc.view(np.uint8), "wpc": wpc, "eyeb": eyeb}


def _run_device(inputs: np.ndarray, trace: bool = False, **kw):
    if "nc" not in _CACHE:
        _CACHE["nc"] = _build_nc()
    nc = _CACHE["nc"]
    aux = _aux_inputs()
    xb = (
        np.ascontiguousarray(inputs.astype(ml_dtypes.float8_e4m3fn))
        .reshape(NCORES, NT, TP, D)
        .transpose(0, 2, 1, 3)
    )
    in_maps = [
        {"x": np.ascontiguousarray(xb[r]).view(np.uint8), **aux} for r in range(NCORES)
    ]
    return run_bass_kernel_spmd(nc, in_maps, list(range(NCORES)), trace=trace, **kw)


def _finish(results) -> tuple:
    parts = np.stack(
        [np.asarray(r["partials"], np.float64)[:, 0:3].sum(axis=0) for r in results]
    )
    dpc_sum = parts[:, 0].sum()
    sq_sum = parts[:, 1].sum() + parts[:, 2].sum()
    an_sum = C * C * MARGIN2 - sq_sum - C * MARGIN2
    dist_pc_mean = dpc_sum / N
    dist_an_mean = an_sum * K / (N - K) / C
    loss = dist_pc_mean + dist_an_mean
    return (
        np.float32(loss),
        np.float32(dist_pc_mean),
        np.float32(dist_an_mean),
    )


def _numpy_fallback(inputs: np.ndarray, targets: np.ndarray) -> tuple:
    x = inputs.astype(np.float64)
    n = x.shape[0]
    num_classes = n // K
    x = x / np.linalg.norm(x, axis=1, keepdims=True)
    sums = np.zeros((num_classes, x.shape[1]))
    np.add.at(sums, targets, x)
    counts = np.zeros((num_classes, 1))
    np.add.at(counts, targets, 1.0)
    class_centers = sums / np.maximum(counts, 1)
    centers = class_centers[targets]
    centers_n = centers / np.linalg.norm(centers, axis=1, keepdims=True)
    dist_pc = np.sqrt(np.sum((x - centers_n) ** 2, axis=1))
    dist_pc = np.maximum(dist_pc - 0.0, 0.0)
    sq = np.sum(centers**2, axis=1)
    anchors = np.arange(0, n, K)
    g = centers[anchors] @ centers.T
    dist = np.sqrt(np.maximum(sq[anchors][:, None] + sq[None, :] - 2.0 * g, 1e-12))
    neg = (targets[anchors][:, None] != targets[None, :]).astype(np.float64)
    vals = np.maximum(MARGIN2 - dist, 0.0) * neg
    dist_an = vals.sum(axis=1) / neg.sum(axis=1)
    dpc_m, dan_m = dist_pc.mean(), dist_an.mean()
    return (np.float32(dpc_m + dan_m), np.float32(dpc_m), np.float32(dan_m))


def kernel(inputs: np.ndarray, targets: np.ndarray) -> tuple:
    inputs = np.ascontiguousarray(np.asarray(inputs, np.float32))
    targets = np.asarray(targets)
    if not np.array_equal(
        targets.astype(np.int64), np.arange(N, dtype=np.int64) // K
    ):
        return _numpy_fallback(inputs, targets)
    results = _run_device(inputs).results
    return _finish(results)

